# revision 21
# baseline (speedup 1.0000x reference)
"""Trainium2 Bass kernel for the 6-layer BigramLanguageModel (B=2, T=1024,
C=1024, H=16, FFN=4096, V=32000).

Strategy: context-parallel over 8 NeuronCores. Cores 0-3 handle batch 0,
cores 4-7 batch 1; each core owns a contiguous 256-token chunk. Per layer
each core computes LN/QKV/attention/proj/FFN for its own tokens; the only
communication is a 4-rank AllGather of K/V per layer (fp16, two feature
halves so attention on the first half overlaps the gather of the second)
plus a final 8-rank AllGather of the pre-lm_head hidden states, split in
two token-halves so the first lm_head pass overlaps the second gather.

All GEMM operands are fp16 (weights pre-cast on host, activations cast on
write); accumulation stays fp32 in PSUM and the residual stream stays
fp32. fp16 weights halve HBM traffic vs f32r and enable FWL fast weight
load, keeping the PE clock warm. The causal mask is added into the scores
PSUM by an identity-weight matmul (tensor engine) so softmax needs only
one wide EXP per key block covering both heads of a head pair. LayerNorm
stats run as fp16 ones-matmuls over [x, x^2]; LN scale/bias are folded
into the consuming weights on host. The lm_head is vocab-sharded 8 ways,
vocab-major (lhsT = W_lm tiles, M=125), two passes of 1024 tokens each.
"""

import os

import numpy as np

# model dims (fixed by the problem)
B, T, V, C, H, HS, L, F = 2, 1024, 32000, 1024, 16, 64, 6, 4096
P = 128
NCORES = 8
GROUP = 4            # cores per batch (context-parallel degree)
NT = T // GROUP      # 256 tokens per core
CCH = C // P         # 8 feature chunks
FCH = F // P         # 32 FFN hidden chunks
KB = T // P          # 8 key blocks
VSH = V // NCORES    # 4000 vocab columns per core
MLM = 32             # lm_head output-partition chunks
MV = VSH // MLM      # 125 vocab rows per chunk
TKH = NT // 2        # 128 tokens per final-AG half per rank
TKW = 512            # lm_head token-chunk width
EPS = 1e-5
NEG = -30000.0

_CACHE = {}


def _build():
    import concourse.bass as bass
    import concourse.tile as tile
    from concourse import bacc, mybir

    f32 = mybir.dt.float32
    f32r = mybir.dt.float32r
    f16 = mybir.dt.float16
    AFT = mybir.ActivationFunctionType
    ALU = mybir.AluOpType

    nc = bacc.Bacc("TRN2", target_bir_lowering=False, debug=False,
                   num_devices=NCORES)

    def din(name, shape, dt=f16):
        return nc.dram_tensor(name, shape, dt, kind="ExternalInput").ap()

    d_x0 = din("x0t", [C, NT], f32)
    d_wq = din("wq", [L, CCH, P, CCH, P])
    d_wk = din("wk", [L, CCH, P, CCH, P])
    d_wo = din("wo", [L, CCH, P, CCH, P])
    d_w1 = din("w1", [L, FCH, P, CCH, P])
    d_w2 = din("w2", [L, CCH, P, FCH, P])
    d_wv = din("wv", [L, C, C])
    d_wlm = din("wlm", [MLM, P, CCH, MV])
    d_qb = din("qb", [L, P, CCH], f32)
    d_kb = din("kb", [L, P, CCH], f32)
    d_b1 = din("b1", [L, P, FCH], f32)
    d_bo = din("bo", [L, C])
    d_b2 = din("b2", [L, C])
    d_vb = din("vb", [L, C], f32)
    d_blm = din("blm", [P, MLM], f32)
    d_msk = din("msk", [KB, P, 2, NT])
    d_on16 = din("on16", [P, NT])
    d_id = din("id16", [P, P])

    d_out = nc.dram_tensor("logits", [VSH, NCORES * NT], f32,
                           kind="ExternalOutput").ap()

    groups4 = [[0, 1, 2, 3], [4, 5, 6, 7]]
    groups8 = [list(range(NCORES))]

    with tile.TileContext(nc) as tc:
        with tc.tile_pool(name="persist", bufs=1) as pp, \
             tc.tile_pool(name="act", bufs=1) as ap_, \
             tc.tile_pool(name="wp", bufs=3) as wp, \
             tc.tile_pool(name="sm", bufs=2) as smp, \
             tc.tile_pool(name="ps", bufs=2, space="PSUM") as psp, \
             tc.tile_pool(name="dram", bufs=2, space="DRAM") as dp:

            # ---------------- persistent tiles ----------------
            x = pp.tile([P, CCH, NT], f32)          # residual stream
            msk2 = pp.tile([P, KB, 2, NT], f16)     # causal mask, both heads
            on16 = pp.tile([P, NT], f16)
            ident = pp.tile([P, P], f16)
            eps_t = pp.tile([1, 1], f32)
            blm_sb = pp.tile([P, MLM], f32)

            nc.sync.dma_start(x[:], d_x0.rearrange("(c p) t -> p c t", p=P))
            nc.sync.dma_start(msk2[:], d_msk.rearrange("k p a t -> p k a t"))
            nc.sync.dma_start(on16[:], d_on16[:])
            nc.sync.dma_start(ident[:], d_id[:])
            nc.sync.dma_start(blm_sb[:], d_blm[:])
            nc.vector.memset(eps_t[:], EPS)

            def layernorm(src, dst):
                """dst (f16) = (src - mu) / sqrt(var + eps), feature-major."""
                ps_st = psp.tile([1, 2 * NT], f32, name="ps_st", tag="sc")
                xh = ap_.tile([P, CCH, NT], f16, name="xh", tag="xh")
                for c in range(CCH):
                    sq = smp.tile([P, NT], f16, name="sq", tag="sq", bufs=3)
                    nc.vector.tensor_copy(xh[:, c, :], src[:, c, :])
                    nc.vector.tensor_mul(sq[:], xh[:, c, :], xh[:, c, :])
                    nc.tensor.matmul(ps_st[:, 0:NT], on16[:, 0:1],
                                     xh[:, c, :],
                                     start=(c == 0), stop=False)
                    nc.tensor.matmul(ps_st[:, NT:2 * NT], on16[:, 0:1],
                                     sq[:],
                                     start=False, stop=(c == CCH - 1))
                st = smp.tile([1, 4, NT], f32, name="st_s", tag="st_s")
                mu = st[:, 0, :]
                ex2 = st[:, 1, :]
                var = st[:, 2, :]
                sd = st[:, 3, :]
                nc.scalar.mul(mu, ps_st[:, 0:NT], 1.0 / C)
                nc.scalar.mul(ex2, ps_st[:, NT:2 * NT], 1.0 / C)
                nc.vector.tensor_mul(var, mu, mu)
                nc.vector.tensor_sub(var, ex2, var)
                nc.scalar.activation(sd, var, AFT.Sqrt, bias=eps_t[:])
                rsrc = smp.tile([1, 2 * NT], f16, name="rsrc", tag="rsrc")
                with nc.allow_low_precision(reason="LN rstd in f16"):
                    nc.vector.reciprocal(rsrc[:, 0:NT], sd)
                    nc.vector.tensor_mul(rsrc[:, NT:2 * NT], mu, rsrc[:, 0:NT])
                ps_rep = psp.tile([P, 2 * NT], f32, name="ps_rep", tag="sc")
                nc.tensor.matmul(ps_rep[:], on16[0:1, 0:P], rsrc[:],
                                 start=True, stop=True)
                rep = smp.tile([P, 2, NT], f16, name="rep", tag="rep")
                nc.scalar.copy(rep[:], ps_rep[:])
                for c in range(CCH):
                    t1 = smp.tile([P, NT], f16, name="t1", tag="t1", bufs=3)
                    nc.vector.tensor_mul(t1[:], xh[:, c, :], rep[:, 0, :])
                    nc.vector.tensor_sub(dst[:, c, :], t1[:], rep[:, 1, :])

            def kv_quarter(l, qi, h, kb_sb, vb_bc):
                """Compute K^T and V for one feature quarter and AllGather."""
                kT = ap_.tile([P, 2, NT], f16, name="kT", tag="kT", bufs=2)
                vT = ap_.tile([P, 2, 256], f16, name="vT", tag="vT", bufs=2)
                for m in range(2 * qi, 2 * qi + 2):
                    wt = wp.tile([P, CCH, P], f16, name="wt_k", tag="wA",
                                 bufs=12)
                    nc.sync.dma_start(wt[:], d_wk[l, m])
                    ps = psp.tile([P, 2 * NT], f32, name="ps_k", tag="mm",
                                  bufs=4)
                    for c in range(CCH):
                        nc.tensor.matmul(ps[:, 0:NT], wt[:, c, :], h[:, c, :],
                                         start=(c == 0), stop=(c == CCH - 1))
                    nc.vector.tensor_scalar(kT[:, m - 2 * qi, :],
                                            ps[:, 0:NT],
                                            kb_sb[:, m:m + 1], None, ALU.add)
                wv_t = wp.tile([P, CCH, 256], f16, name="wv_t", tag="wB",
                               bufs=4)
                nc.sync.dma_start(
                    wv_t[:],
                    d_wv[l].rearrange("(c p) f -> p c f", p=P)
                    [:, :, qi * 256:(qi + 1) * 256])
                for tk in range(2):
                    ps = psp.tile([P, 2 * NT], f32, name="ps_v", tag="mm",
                                  bufs=4)
                    for c in range(CCH):
                        nc.tensor.matmul(
                            ps[:, 0:256], h[:, c, tk * P:(tk + 1) * P],
                            wv_t[:, c, :], start=(c == 0), stop=(c == CCH - 1))
                    nc.vector.tensor_add(
                        vT[:, tk, :], ps[:, 0:256],
                        vb_bc[:, qi * 256:(qi + 1) * 256])
                kv_in = dp.tile([2, 256 * NT], f16, name="kv_in",
                                tag=f"kv_in{qi}")
                kv_out = dp.tile([GROUP, 2, 256 * NT], f16, name="kv_out",
                                 tag=f"kv_out{qi}")
                nc.sync.dma_start(
                    kv_in[0].rearrange("(c t) -> c t", t=NT)
                    .rearrange("(c p) t -> p c t", p=P), kT[:])
                nc.sync.dma_start(
                    kv_in[1].rearrange("(g f) -> g f", f=256)
                    .rearrange("(g p) f -> p g f", p=P), vT[:])
                nc.gpsimd.collective_compute(
                    "AllGather", mybir.AluOpType.bypass,
                    replica_groups=groups4,
                    ins=[kv_in.opt()], outs=[kv_out.opt()])
                return kv_out

            # ================= transformer layers =================
            for l in range(L):
                # ---- LN1 ----
                h = ap_.tile([P, CCH, NT], f16, name="h", tag="h")
                layernorm(x, h)

                qb_sb = smp.tile([P, CCH], f32, name="qb_sb", tag="qb")
                kb_sb = smp.tile([P, CCH], f32, name="kb_sb", tag="kb")
                nc.sync.dma_start(qb_sb[:], d_qb[l])
                nc.sync.dma_start(kb_sb[:], d_kb[l])
                vb_bc = smp.tile([P, C], f32, name="vb_bc", tag="vb")
                vb_l = d_vb[l]
                nc.gpsimd.dma_start(
                    vb_bc[:],
                    bass.AP(tensor=vb_l.tensor, offset=vb_l.offset,
                            ap=[[0, P], *vb_l.ap]))

                # ---- K/V per feature quarter, AllGather each ----
                kv_outs = []
                for qi in range(4):
                    kv_outs.append(kv_quarter(l, qi, h, kb_sb, vb_bc))

                # ---- Q ----
                qT = ap_.tile([P, CCH, NT], f16, name="qT", tag="qT")
                for m in range(CCH):
                    wt = wp.tile([P, CCH, P], f16, name="wt_q", tag="wA",
                                 bufs=12)
                    nc.sync.dma_start(wt[:], d_wq[l, m])
                    ps = psp.tile([P, 2 * NT], f32, name="ps_q", tag="mm",
                                  bufs=4)
                    for c in range(CCH):
                        nc.tensor.matmul(ps[:, 0:NT], wt[:, c, :], h[:, c, :],
                                         start=(c == 0), stop=(c == CCH - 1))
                    nc.vector.tensor_scalar(qT[:, m, :], ps[:, 0:NT],
                                            qb_sb[:, m:m + 1], None, ALU.add)

                # ---- attention (16 heads; head pair hp shares K/V tiles) ----
                attn_hm = ap_.tile([HS, H, NT], f16, name="attn_hm",
                                   tag="ahm")
                for hp in range(H // 2):
                    kvo = kv_outs[hp // 2]
                    hpl = hp % 2
                    KT_hp = smp.tile([P, GROUP * NT], f16, name="KT_hp",
                                     tag="KT_hp", bufs=3)
                    for r in range(GROUP):
                        nc.gpsimd.dma_start(
                            KT_hp[:, r * NT:(r + 1) * NT],
                            kvo[r, 0].rearrange("(c t) -> c t", t=NT)
                            [hpl * P:(hpl + 1) * P, :])
                    vaug = smp.tile([P, KB, 2, HS + 1], f16, name="vaug",
                                    tag="vaug", bufs=3)
                    for gg in range(KB):
                        r, j = gg // 2, gg % 2
                        nc.gpsimd.dma_start(
                            vaug[:, gg, :, 0:HS],
                            kvo[r, 1].rearrange("(t f) -> t f", f=256)
                            [j * P:(j + 1) * P,
                             hpl * 2 * HS:(hpl + 1) * 2 * HS]
                            .rearrange("p (a b) -> p a b", a=2))
                    nc.vector.tensor_copy(
                        vaug[:, :, :, HS],
                        on16[:, 0:2 * KB].rearrange("p (a b) -> p a b", a=KB))

                    for b in range(2):
                        hh = 2 * hp + b
                        po = psp.tile([P, NT], f32, name="po", tag="po",
                                      bufs=2)
                        for g in range(KB):
                            ps = psp.tile([P, NT], f32, name="ps_s", tag="sc")
                            nc.tensor.matmul(
                                ps[:],
                                KT_hp[b * HS:(b + 1) * HS, g * P:(g + 1) * P],
                                qT[b * HS:(b + 1) * HS, hp, :],
                                start=True, stop=True)
                            sm = smp.tile([P, NT], f32, name="sm_t",
                                          tag="sm_t", bufs=4)
                            nc.vector.tensor_add(sm[:], ps[:],
                                                 msk2[:, g, 0, :])
                            e = smp.tile([P, NT], f16, name="e", tag="e",
                                         bufs=6)
                            nc.scalar.activation(e[:], sm[:], AFT.Exp)
                            nc.tensor.matmul(po[0:HS + 1, :], vaug[:, g, b, :],
                                             e[:], start=(g == 0),
                                             stop=(g == KB - 1))
                        rec = smp.tile([P, NT], f16, name="rec", tag="rec")
                        with nc.allow_low_precision(reason="softmax recip"):
                            nc.vector.reciprocal(rec[HS:HS + 1, :],
                                                 po[HS:HS + 1, :])
                        prep = psp.tile([HS, NT], f32, name="prep", tag="sc")
                        nc.tensor.matmul(prep[:], on16[HS:HS + 1, 0:HS],
                                         rec[HS:HS + 1, :],
                                         start=True, stop=True)
                        rep_s = smp.tile([HS, NT], f16, name="rep_s",
                                         tag="reps")
                        nc.scalar.copy(rep_s[:], prep[:])
                        nc.vector.tensor_mul(attn_hm[:, hh, :], po[0:HS, :],
                                             rep_s[:])

                # reshape heads back to feature-major [P, CCH, NT]
                attn2 = ap_.tile([P, CCH, NT], f16, name="attn2", tag="at2")
                ahm4 = attn_hm.rearrange("p (m b) t -> p m b t", b=2)
                nc.sync.dma_start(attn2[0:HS, :, :], ahm4[:, :, 0, :])
                nc.sync.dma_start(attn2[HS:P, :, :], ahm4[:, :, 1, :])

                # ---- output projection + residual ----
                bo_sb = smp.tile([1, C], f16, name="bo_sb", tag="bo")
                nc.sync.dma_start(bo_sb[:], d_bo[l][None, :])
                for m in range(CCH):
                    wt = wp.tile([P, CCH, P], f16, name="wt_o", tag="wA",
                                 bufs=12)
                    nc.sync.dma_start(wt[:], d_wo[l, m])
                    ps = psp.tile([P, 2 * NT], f32, name="ps_o", tag="mm",
                                  bufs=4)
                    for c in range(CCH):
                        nc.tensor.matmul(ps[:, 0:NT], wt[:, c, :],
                                         attn2[:, c, :],
                                         start=(c == 0), stop=False)
                    nc.tensor.matmul(ps[:, 0:NT],
                                     bo_sb[:, m * P:(m + 1) * P],
                                     on16[0:1, :],
                                     start=False, stop=True)
                    nc.vector.tensor_add(x[:, m, :], ps[:, 0:NT], x[:, m, :])

                # ---- LN2 + FFN ----
                h2 = ap_.tile([P, CCH, NT], f16, name="h2", tag="h")
                layernorm(x, h2)

                b1_sb = smp.tile([P, FCH], f32, name="b1_sb", tag="qb")
                nc.sync.dma_start(b1_sb[:], d_b1[l])
                hid = ap_.tile([P, FCH, NT], f16, name="hid", tag="hid")
                for fh in range(FCH):
                    wt = wp.tile([P, CCH, P], f16, name="wt_1", tag="wA",
                                 bufs=12)
                    nc.sync.dma_start(wt[:], d_w1[l, fh])
                    ps = psp.tile([P, 2 * NT], f32, name="ps_1", tag="mm",
                                  bufs=4)
                    for c in range(CCH):
                        nc.tensor.matmul(ps[:, 0:NT], wt[:, c, :], h2[:, c, :],
                                         start=(c == 0), stop=(c == CCH - 1))
                    nc.vector.tensor_scalar(hid[:, fh, :], ps[:, 0:NT],
                                            b1_sb[:, fh:fh + 1], 0.0,
                                            ALU.add, ALU.max)

                b2_sb = smp.tile([1, C], f16, name="b2_sb", tag="bo")
                nc.sync.dma_start(b2_sb[:], d_b2[l][None, :])
                for m in range(CCH):
                    wt2 = wp.tile([P, FCH, P], f16, name="wt_2", tag="wB",
                                  bufs=4)
                    nc.sync.dma_start(wt2[:], d_w2[l, m])
                    ps = psp.tile([P, 2 * NT], f32, name="ps_2", tag="mm",
                                  bufs=4)
                    for fc in range(FCH):
                        nc.tensor.matmul(ps[:, 0:NT], wt2[:, fc, :],
                                         hid[:, fc, :],
                                         start=(fc == 0), stop=False)
                    nc.tensor.matmul(ps[:, 0:NT],
                                     b2_sb[:, m * P:(m + 1) * P],
                                     on16[0:1, :],
                                     start=False, stop=True)
                    nc.vector.tensor_add(x[:, m, :], ps[:, 0:NT], x[:, m, :])

            # ================= final LN + lm_head =================
            zf = ap_.tile([P, CCH, NT], f16, name="zf", tag="h")
            layernorm(x, zf)

            xf_outs = []
            for ha in range(2):
                xf_in = dp.tile([C, TKH], f16, name="xf_in",
                                tag=f"xf_in{ha}", bufs=1)
                xf_out = dp.tile([NCORES, C * TKH], f16, name="xf_out",
                                 tag=f"xf_out{ha}", bufs=1,
                                 addr_space="Shared")
                nc.sync.dma_start(
                    xf_in.rearrange("(c p) t -> p c t", p=P),
                    zf[:, :, ha * TKH:(ha + 1) * TKH])
                nc.gpsimd.collective_compute(
                    "AllGather", mybir.AluOpType.bypass,
                    replica_groups=groups8,
                    ins=[xf_in.opt()], outs=[xf_out.opt()])
                xf_outs.append(xf_out)

            # two passes: pass ha covers AG half ha = 1024 tokens (2 chunks
            # of 512); W_lm tiles are loaded once per pass.
            for ha in range(2):
                xf_n = []
                for rg in range(2):
                    xt = wp.tile([P, CCH, TKW], f16, name="xf_n",
                                 tag=f"xf{rg}", bufs=2)
                    for rl in range(GROUP):
                        nc.sync.dma_start(
                            xt[:, :, rl * TKH:(rl + 1) * TKH],
                            xf_outs[ha][rg * GROUP + rl]
                            .rearrange("(c p t) -> p c t", p=P, t=TKH))
                    xf_n.append(xt)
                for m in range(MLM):
                    wlm_t = wp.tile([P, CCH, MV], f16, name="wlm_t",
                                    tag="wA", bufs=12)
                    nc.sync.dma_start(wlm_t[:], d_wlm[m])
                    for rg in range(2):
                        n = 2 * ha + rg
                        ps = psp.tile([P, 2 * NT], f32, name="ps_lm",
                                      tag="mm", bufs=4)
                        for c in range(CCH):
                            nc.tensor.matmul(ps[0:MV, 0:TKW], wlm_t[:, c, :],
                                             xf_n[rg][:, c, :],
                                             start=(c == 0),
                                             stop=(c == CCH - 1))
                        lo = smp.tile([P, TKW], f32, name="lo", tag="lo",
                                      bufs=3)
                        nc.vector.tensor_scalar(lo[0:MV, :], ps[0:MV, 0:TKW],
                                                blm_sb[0:MV, m:m + 1], None,
                                                ALU.add)
                        nc.sync.dma_start(
                            d_out[m * MV:(m + 1) * MV,
                                  n * TKW:(n + 1) * TKW],
                            lo[0:MV, :])

    nc.compile()
    return nc


def _host_prep(inputs):
    """Fold LN scale/bias into weights, pre-tile lhsT weights, build masks."""
    f = np.float32
    h16 = np.float16
    g = {k: np.asarray(v) for k, v in inputs.items()}

    tok_emb = g["tok_emb"].astype(f)
    pos_emb = g["pos_emb"].astype(f)
    idx = np.asarray(g["idx"]).astype(np.int64)

    x0 = tok_emb[idx] + pos_emb[None, :T, :]          # [B, T, C]

    def cat_heads(w):                                  # [H, C, HS] -> [C, H*HS]
        return np.ascontiguousarray(w.transpose(1, 0, 2).reshape(C, H * HS))

    def tile_lhst(w, nm):
        # [Cin, Cout] -> [Cout/nm-chunks (m), P(p over Cin), Cin/P (c), f]
        cin = w.shape[0]
        r = w.reshape(cin // P, P, nm, w.shape[1] // nm)   # [c, p, m, f]
        return np.ascontiguousarray(
            r.transpose(2, 1, 0, 3).astype(h16))           # [m, p, c, f]

    wq = np.empty((L, CCH, P, CCH, P), h16)
    wk = np.empty((L, CCH, P, CCH, P), h16)
    wo = np.empty((L, CCH, P, CCH, P), h16)
    w1 = np.empty((L, FCH, P, CCH, P), h16)
    w2 = np.empty((L, CCH, P, FCH, P), h16)
    wv = np.empty((L, C, C), h16)
    qb = np.empty((L, P, CCH), f)
    kb = np.empty((L, P, CCH), f)
    b1t = np.empty((L, P, FCH), f)
    vb = np.empty((L, C), f)

    scale = 1.0 / np.sqrt(HS)
    for l in range(L):
        s1 = g["ln1_s"][l].astype(f)
        bn1 = g["ln1_b"][l].astype(f)
        s2 = g["ln2_s"][l].astype(f)
        bn2 = g["ln2_b"][l].astype(f)
        Wq = cat_heads(g["Wq"][l].astype(f))
        Wk = cat_heads(g["Wk"][l].astype(f))
        Wv = cat_heads(g["Wv"][l].astype(f))
        wq[l] = tile_lhst(s1[:, None] * Wq * scale, CCH)
        wk[l] = tile_lhst(s1[:, None] * Wk, CCH)
        wo[l] = tile_lhst(g["Wo"][l].astype(f), CCH)
        wv[l] = (s1[:, None] * Wv).astype(h16)
        qb[l] = ((bn1 @ Wq) * scale).reshape(CCH, P).T
        kb[l] = (bn1 @ Wk).reshape(CCH, P).T
        vb[l] = bn1 @ Wv
        W1 = g["W1"][l].astype(f)
        w1[l] = tile_lhst(s2[:, None] * W1, FCH)
        b1t[l] = (g["b1"][l].astype(f) + bn2 @ W1).reshape(FCH, P).T
        w2[l] = tile_lhst(g["W2"][l].astype(f), CCH)

    sf = g["lnf_s"].astype(f)
    bf = g["lnf_b"].astype(f)
    Wlm = g["W_lm"].astype(f)
    wlm_full = sf[:, None] * Wlm                      # [C, V]
    blm_full = (g["b_lm"].astype(f) + bf @ Wlm)       # [V]

    on16 = np.ones((P, NT), h16)
    id16 = np.eye(P, dtype=h16)

    shared = dict(
        wq=wq, wk=wk, wo=wo, w1=w1, w2=w2, wv=wv,
        qb=np.ascontiguousarray(qb), kb=np.ascontiguousarray(kb),
        b1=np.ascontiguousarray(b1t),
        bo=g["bo"].astype(h16), b2=g["b2"].astype(h16), vb=vb,
        on16=on16, id16=id16,
    )

    in_maps = []
    for core in range(NCORES):
        bb, cg = core // GROUP, core % GROUP
        x0t = np.ascontiguousarray(
            x0[bb, cg * NT:(cg + 1) * NT, :].T)       # [C, NT]
        qpos = cg * NT + np.arange(NT)
        kpos = np.arange(T)
        mask = np.where(kpos[:, None] <= qpos[None, :], 0.0, NEG).astype(h16)
        mask = mask.reshape(KB, P, NT)
        msk2 = np.ascontiguousarray(
            np.broadcast_to(mask[:, :, None, :], (KB, P, 2, NT)))
        wlm_s = wlm_full[:, core * VSH:(core + 1) * VSH]   # [C, 4000]
        blm_s = blm_full[core * VSH:(core + 1) * VSH]
        blm_t = np.zeros((P, MLM), f)
        blm_t[:MV, :] = blm_s.reshape(MLM, MV).T
        m = dict(shared)
        m["x0t"] = x0t
        m["msk"] = msk2
        m["wlm"] = tile_lhst(wlm_s, MLM)              # [32, 128, 8, 125]
        m["blm"] = blm_t
        in_maps.append(m)
    return in_maps


def _token_cols():
    """Global flat-token index for each column of the device output."""
    cols = np.empty(NCORES * NT, np.int64)
    i = 0
    for ha in range(2):
        for rg in range(2):
            for rl in range(GROUP):
                r = rg * GROUP + rl
                bb, cg = divmod(r, GROUP)
                base = bb * T + cg * NT + ha * TKH
                cols[i:i + TKH] = np.arange(base, base + TKH)
                i += TKH
    return cols


def kernel(**inputs):
    from concourse import bass_utils

    if "nc" not in _CACHE:
        _CACHE["nc"] = _build()
    nc = _CACHE["nc"]

    in_maps = _host_prep(inputs)
    trace = os.environ.get("BIGRAM_TRACE") == "1"
    res = bass_utils.run_bass_kernel_spmd(
        nc, in_maps, core_ids=list(range(NCORES)), trace=trace)
    _CACHE["last_res"] = res

    cols = _token_cols()
    out = np.empty((B * T, V), np.float32)
    for core in range(NCORES):
        out[cols, core * VSH:(core + 1) * VSH] = res.results[core]["logits"].T
    return out.reshape(B, T, V)


# revision 22
# speedup vs baseline: 1.0132x; 1.0132x over previous
"""Trainium2 Bass kernel for the 6-layer BigramLanguageModel (B=2, T=1024,
C=1024, H=16, FFN=4096, V=32000).

Strategy: context-parallel over 8 NeuronCores. Cores 0-3 handle batch 0,
cores 4-7 batch 1; each core owns a contiguous 256-token chunk. Per layer
each core computes LN/QKV/attention/proj/FFN for its own tokens; the only
communication is four 4-rank AllGathers of K/V feature-quarters per layer
(fp16; quarter q feeds head pairs 2q,2q+1, so attention on early quarters
overlaps the gather of later ones) plus a final 8-rank AllGather of the
pre-lm_head hidden states, split in two token-halves so the first lm_head
pass overlaps the second gather.

All GEMM operands are fp16 (weights pre-cast on host, activations cast on
write); accumulation stays fp32 in PSUM and the residual stream stays
fp32. fp16 weights halve HBM traffic vs f32r and enable FWL fast weight
load. LayerNorm stats run as fp16 ones-matmuls over x and x^2; LN
scale/bias are folded into the consuming weights on host. Softmax skips
the max-subtraction (scores are O(1) by construction); the causal mask is
per-core input data added on the vector engine; the denominator comes
from augmenting V with a ones column (M=65 PV matmuls). The lm_head is
vocab-sharded 8 ways, vocab-major (lhsT = W_lm tiles, M=125), two passes
of 1024 tokens each with W_lm tiles loaded once per pass.
"""

import os

import numpy as np

# model dims (fixed by the problem)
B, T, V, C, H, HS, L, F = 2, 1024, 32000, 1024, 16, 64, 6, 4096
P = 128
NCORES = 8
GROUP = 4            # cores per batch (context-parallel degree)
NT = T // GROUP      # 256 tokens per core
CCH = C // P         # 8 feature chunks
FCH = F // P         # 32 FFN hidden chunks
KB = T // P          # 8 key blocks
VSH = V // NCORES    # 4000 vocab columns per core
MLM = 32             # lm_head output-partition chunks
MV = VSH // MLM      # 125 vocab rows per chunk
TKH = NT // 2        # 128 tokens per final-AG half per rank
TKW = 512            # lm_head token-chunk width
EPS = 1e-5
NEG = -30000.0

_CACHE = {}


def _build():
    import concourse.bass as bass
    import concourse.tile as tile
    from concourse import bacc, mybir

    f32 = mybir.dt.float32
    f32r = mybir.dt.float32r
    f16 = mybir.dt.float16
    AFT = mybir.ActivationFunctionType
    ALU = mybir.AluOpType

    nc = bacc.Bacc("TRN2", target_bir_lowering=False, debug=False,
                   num_devices=NCORES)

    def din(name, shape, dt=f16):
        return nc.dram_tensor(name, shape, dt, kind="ExternalInput").ap()

    d_x0 = din("x0t", [C, NT], f32)
    d_wq = din("wq", [L, CCH, P, CCH, P])
    d_wk = din("wk", [L, CCH, P, CCH, P])
    d_wo = din("wo", [L, CCH, P, CCH, P])
    d_w1 = din("w1", [L, FCH, P, CCH, P])
    d_w2 = din("w2", [L, CCH, P, FCH, P])
    d_wv = din("wv", [L, C, C])
    d_wlm = din("wlm", [MLM, P, CCH, MV])
    d_qb = din("qb", [L, P, CCH], f32)
    d_kb = din("kb", [L, P, CCH], f32)
    d_b1 = din("b1", [L, P, FCH], f32)
    d_bo = din("bo", [L, C])
    d_b2 = din("b2", [L, C])
    d_vb = din("vb", [L, C], f32)
    d_blm = din("blm", [P, MLM], f32)
    d_msk = din("msk", [KB, P, 2, NT])
    d_on16 = din("on16", [P, NT])
    d_id = din("id16", [P, P])

    d_out = nc.dram_tensor("logits", [VSH, NCORES * NT], f32,
                           kind="ExternalOutput").ap()

    groups4 = [[0, 1, 2, 3], [4, 5, 6, 7]]
    groups8 = [list(range(NCORES))]

    with tile.TileContext(nc) as tc:
        with tc.tile_pool(name="persist", bufs=1) as pp, \
             tc.tile_pool(name="act", bufs=1) as ap_, \
             tc.tile_pool(name="wp", bufs=3) as wp, \
             tc.tile_pool(name="sm", bufs=2) as smp, \
             tc.tile_pool(name="ps", bufs=2, space="PSUM") as psp, \
             tc.tile_pool(name="dram", bufs=2, space="DRAM") as dp:

            # ---------------- persistent tiles ----------------
            x = pp.tile([P, CCH, NT], f32)          # residual stream
            msk2 = pp.tile([P, KB, 2, NT], f16)     # causal mask, both heads
            on16 = pp.tile([P, NT], f16)
            ident = pp.tile([P, P], f16)
            eps_t = pp.tile([1, 1], f32)
            blm_sb = pp.tile([P, MLM], f32)

            nc.sync.dma_start(x[:], d_x0.rearrange("(c p) t -> p c t", p=P))
            nc.sync.dma_start(msk2[:], d_msk.rearrange("k p a t -> p k a t"))
            nc.sync.dma_start(on16[:], d_on16[:])
            nc.sync.dma_start(ident[:], d_id[:])
            nc.sync.dma_start(blm_sb[:], d_blm[:])
            nc.vector.memset(eps_t[:], EPS)

            def layernorm(src, dst):
                """dst (f16) = (src - mu) / sqrt(var + eps), feature-major."""
                ps_st = psp.tile([1, 2 * NT], f32, name="ps_st", tag="sc")
                xh = ap_.tile([P, CCH, NT], f16, name="xh", tag="xh")
                for c in range(CCH):
                    sq = smp.tile([P, NT], f16, name="sq", tag="sq", bufs=3)
                    nc.vector.tensor_copy(xh[:, c, :], src[:, c, :])
                    nc.vector.tensor_mul(sq[:], xh[:, c, :], xh[:, c, :])
                    nc.tensor.matmul(ps_st[:, 0:NT], on16[:, 0:1],
                                     xh[:, c, :],
                                     start=(c == 0), stop=False)
                    nc.tensor.matmul(ps_st[:, NT:2 * NT], on16[:, 0:1],
                                     sq[:],
                                     start=False, stop=(c == CCH - 1))
                st = smp.tile([1, 4, NT], f32, name="st_s", tag="st_s")
                mu = st[:, 0, :]
                ex2 = st[:, 1, :]
                var = st[:, 2, :]
                sd = st[:, 3, :]
                nc.scalar.mul(mu, ps_st[:, 0:NT], 1.0 / C)
                nc.scalar.mul(ex2, ps_st[:, NT:2 * NT], 1.0 / C)
                nc.vector.tensor_mul(var, mu, mu)
                nc.vector.tensor_sub(var, ex2, var)
                nc.scalar.activation(sd, var, AFT.Sqrt, bias=eps_t[:])
                rsrc = smp.tile([1, 2 * NT], f16, name="rsrc", tag="rsrc")
                with nc.allow_low_precision(reason="LN rstd in f16"):
                    nc.vector.reciprocal(rsrc[:, 0:NT], sd)
                    nc.vector.tensor_mul(rsrc[:, NT:2 * NT], mu, rsrc[:, 0:NT])
                ps_rep = psp.tile([P, 2 * NT], f32, name="ps_rep", tag="sc")
                nc.tensor.matmul(ps_rep[:], on16[0:1, 0:P], rsrc[:],
                                 start=True, stop=True)
                rep = smp.tile([P, 2, NT], f16, name="rep", tag="rep")
                nc.scalar.copy(rep[:], ps_rep[:])
                for c in range(CCH):
                    t1 = smp.tile([P, NT], f16, name="t1", tag="t1", bufs=3)
                    nc.vector.tensor_mul(t1[:], xh[:, c, :], rep[:, 0, :])
                    nc.vector.tensor_sub(dst[:, c, :], t1[:], rep[:, 1, :])

            def kv_quarter(l, qi, h, kb_sb, vb_bc):
                """Compute K^T and V for one feature quarter and AllGather."""
                kT = ap_.tile([P, 2, NT], f16, name="kT", tag="kT", bufs=2)
                vT = ap_.tile([P, 2, 256], f16, name="vT", tag="vT", bufs=2)
                for m in range(2 * qi, 2 * qi + 2):
                    wt = wp.tile([P, CCH, P], f16, name="wt_k", tag="wA",
                                 bufs=8)
                    nc.sync.dma_start(wt[:], d_wk[l, m])
                    ps = psp.tile([P, 2 * NT], f32, name="ps_k", tag="mm",
                                  bufs=3)
                    for c in range(CCH):
                        nc.tensor.matmul(ps[:, 0:NT], wt[:, c, :], h[:, c, :],
                                         start=(c == 0), stop=(c == CCH - 1))
                    nc.vector.tensor_scalar(kT[:, m - 2 * qi, :],
                                            ps[:, 0:NT],
                                            kb_sb[:, m:m + 1], None, ALU.add)
                wv_t = wp.tile([P, CCH, 256], f16, name="wv_t", tag="wB",
                               bufs=3)
                nc.sync.dma_start(
                    wv_t[:],
                    d_wv[l].rearrange("(c p) f -> p c f", p=P)
                    [:, :, qi * 256:(qi + 1) * 256])
                for tk in range(2):
                    ps = psp.tile([P, 2 * NT], f32, name="ps_v", tag="mm",
                                  bufs=3)
                    for c in range(CCH):
                        nc.tensor.matmul(
                            ps[:, 0:256], h[:, c, tk * P:(tk + 1) * P],
                            wv_t[:, c, :], start=(c == 0), stop=(c == CCH - 1))
                    nc.vector.tensor_add(
                        vT[:, tk, :], ps[:, 0:256],
                        vb_bc[:, qi * 256:(qi + 1) * 256])
                kv_in = dp.tile([2, 256 * NT], f16, name="kv_in",
                                tag=f"kv_in{qi}")
                kv_out = dp.tile([GROUP, 2, 256 * NT], f16, name="kv_out",
                                 tag=f"kv_out{qi}")
                nc.sync.dma_start(
                    kv_in[0].rearrange("(c t) -> c t", t=NT)
                    .rearrange("(c p) t -> p c t", p=P), kT[:])
                nc.sync.dma_start(
                    kv_in[1].rearrange("(g f) -> g f", f=256)
                    .rearrange("(g p) f -> p g f", p=P), vT[:])
                nc.gpsimd.collective_compute(
                    "AllGather", mybir.AluOpType.bypass,
                    replica_groups=groups4,
                    ins=[kv_in.opt()], outs=[kv_out.opt()])
                return kv_out

            # ================= transformer layers =================
            for l in range(L):
                # ---- LN1 ----
                h = ap_.tile([P, CCH, NT], f16, name="h", tag="h")
                layernorm(x, h)

                qb_sb = smp.tile([P, CCH], f32, name="qb_sb", tag="qb")
                kb_sb = smp.tile([P, CCH], f32, name="kb_sb", tag="kb")
                nc.sync.dma_start(qb_sb[:], d_qb[l])
                nc.sync.dma_start(kb_sb[:], d_kb[l])
                vb_bc = smp.tile([P, C], f32, name="vb_bc", tag="vb")
                vb_l = d_vb[l]
                nc.gpsimd.dma_start(
                    vb_bc[:],
                    bass.AP(tensor=vb_l.tensor, offset=vb_l.offset,
                            ap=[[0, P], *vb_l.ap]))

                # ---- K/V per feature quarter, AllGather each ----
                kv_outs = []
                for qi in range(4):
                    kv_outs.append(kv_quarter(l, qi, h, kb_sb, vb_bc))

                # ---- Q ----
                qT = ap_.tile([P, CCH, NT], f16, name="qT", tag="qT")
                for m in range(CCH):
                    wt = wp.tile([P, CCH, P], f16, name="wt_q", tag="wA",
                                 bufs=8)
                    nc.sync.dma_start(wt[:], d_wq[l, m])
                    ps = psp.tile([P, 2 * NT], f32, name="ps_q", tag="mm",
                                  bufs=3)
                    for c in range(CCH):
                        nc.tensor.matmul(ps[:, 0:NT], wt[:, c, :], h[:, c, :],
                                         start=(c == 0), stop=(c == CCH - 1))
                    nc.vector.tensor_scalar(qT[:, m, :], ps[:, 0:NT],
                                            qb_sb[:, m:m + 1], None, ALU.add)

                # ---- attention (16 heads; head pair hp shares K/V tiles) ----
                attn_hm = ap_.tile([HS, H, NT], f16, name="attn_hm",
                                   tag="ahm")
                for hp in range(H // 2):
                    kvo = kv_outs[hp // 2]
                    hpl = hp % 2
                    KT_hp = smp.tile([P, GROUP * NT], f16, name="KT_hp",
                                     tag="KT_hp", bufs=2)
                    for r in range(GROUP):
                        nc.gpsimd.dma_start(
                            KT_hp[:, r * NT:(r + 1) * NT],
                            kvo[r, 0].rearrange("(c t) -> c t", t=NT)
                            [hpl * P:(hpl + 1) * P, :])
                    vaug = smp.tile([P, KB, 2, HS + 1], f16, name="vaug",
                                    tag="vaug", bufs=3)
                    for gg in range(KB):
                        r, j = gg // 2, gg % 2
                        nc.gpsimd.dma_start(
                            vaug[:, gg, :, 0:HS],
                            kvo[r, 1].rearrange("(t f) -> t f", f=256)
                            [j * P:(j + 1) * P,
                             hpl * 2 * HS:(hpl + 1) * 2 * HS]
                            .rearrange("p (a b) -> p a b", a=2))
                    nc.vector.tensor_copy(
                        vaug[:, :, :, HS],
                        on16[:, 0:2 * KB].rearrange("p (a b) -> p a b", a=KB))

                    for b in range(2):
                        hh = 2 * hp + b
                        po = psp.tile([P, NT], f32, name="po", tag="po",
                                      bufs=2)
                        for g in range(KB):
                            ps = psp.tile([P, NT], f32, name="ps_s", tag="sc")
                            nc.tensor.matmul(
                                ps[:],
                                KT_hp[b * HS:(b + 1) * HS, g * P:(g + 1) * P],
                                qT[b * HS:(b + 1) * HS, hp, :],
                                start=True, stop=True)
                            sm = smp.tile([P, NT], f32, name="sm_t",
                                          tag="sm_t", bufs=4)
                            nc.vector.tensor_add(sm[:], ps[:],
                                                 msk2[:, g, 0, :])
                            e = smp.tile([P, NT], f16, name="e", tag="e",
                                         bufs=6)
                            nc.scalar.activation(e[:], sm[:], AFT.Exp)
                            nc.tensor.matmul(po[0:HS + 1, :], vaug[:, g, b, :],
                                             e[:], start=(g == 0),
                                             stop=(g == KB - 1))
                        rec = smp.tile([P, NT], f16, name="rec", tag="rec")
                        with nc.allow_low_precision(reason="softmax recip"):
                            nc.vector.reciprocal(rec[HS:HS + 1, :],
                                                 po[HS:HS + 1, :])
                        prep = psp.tile([HS, NT], f32, name="prep", tag="sc")
                        nc.tensor.matmul(prep[:], on16[HS:HS + 1, 0:HS],
                                         rec[HS:HS + 1, :],
                                         start=True, stop=True)
                        rep_s = smp.tile([HS, NT], f16, name="rep_s",
                                         tag="reps")
                        nc.scalar.copy(rep_s[:], prep[:])
                        nc.vector.tensor_mul(attn_hm[:, hh, :], po[0:HS, :],
                                             rep_s[:])

                # reshape heads back to feature-major [P, CCH, NT]
                attn2 = ap_.tile([P, CCH, NT], f16, name="attn2", tag="at2")
                ahm4 = attn_hm.rearrange("p (m b) t -> p m b t", b=2)
                nc.sync.dma_start(attn2[0:HS, :, :], ahm4[:, :, 0, :])
                nc.sync.dma_start(attn2[HS:P, :, :], ahm4[:, :, 1, :])

                # ---- output projection + residual ----
                bo_sb = smp.tile([1, C], f16, name="bo_sb", tag="bo")
                nc.sync.dma_start(bo_sb[:], d_bo[l][None, :])
                for m in range(CCH):
                    wt = wp.tile([P, CCH, P], f16, name="wt_o", tag="wA",
                                 bufs=8)
                    nc.sync.dma_start(wt[:], d_wo[l, m])
                    ps = psp.tile([P, 2 * NT], f32, name="ps_o", tag="mm",
                                  bufs=3)
                    for c in range(CCH):
                        nc.tensor.matmul(ps[:, 0:NT], wt[:, c, :],
                                         attn2[:, c, :],
                                         start=(c == 0), stop=False)
                    nc.tensor.matmul(ps[:, 0:NT],
                                     bo_sb[:, m * P:(m + 1) * P],
                                     on16[0:1, :],
                                     start=False, stop=True)
                    nc.vector.tensor_add(x[:, m, :], ps[:, 0:NT], x[:, m, :])

                # ---- LN2 + FFN ----
                h2 = ap_.tile([P, CCH, NT], f16, name="h2", tag="h")
                layernorm(x, h2)

                b1_sb = smp.tile([P, FCH], f32, name="b1_sb", tag="qb")
                nc.sync.dma_start(b1_sb[:], d_b1[l])
                hid = ap_.tile([P, FCH, NT], f16, name="hid", tag="hid")
                for fh in range(FCH):
                    wt = wp.tile([P, CCH, P], f16, name="wt_1", tag="wA",
                                 bufs=8)
                    nc.sync.dma_start(wt[:], d_w1[l, fh])
                    ps = psp.tile([P, 2 * NT], f32, name="ps_1", tag="mm",
                                  bufs=3)
                    for c in range(CCH):
                        nc.tensor.matmul(ps[:, 0:NT], wt[:, c, :], h2[:, c, :],
                                         start=(c == 0), stop=(c == CCH - 1))
                    nc.vector.tensor_scalar(hid[:, fh, :], ps[:, 0:NT],
                                            b1_sb[:, fh:fh + 1], 0.0,
                                            ALU.add, ALU.max)

                b2_sb = smp.tile([1, C], f16, name="b2_sb", tag="bo")
                nc.sync.dma_start(b2_sb[:], d_b2[l][None, :])
                for m in range(CCH):
                    wt2 = wp.tile([P, FCH, P], f16, name="wt_2", tag="wB",
                                  bufs=3)
                    nc.sync.dma_start(wt2[:], d_w2[l, m])
                    ps = psp.tile([P, 2 * NT], f32, name="ps_2", tag="mm",
                                  bufs=3)
                    for fc in range(FCH):
                        nc.tensor.matmul(ps[:, 0:NT], wt2[:, fc, :],
                                         hid[:, fc, :],
                                         start=(fc == 0), stop=False)
                    nc.tensor.matmul(ps[:, 0:NT],
                                     b2_sb[:, m * P:(m + 1) * P],
                                     on16[0:1, :],
                                     start=False, stop=True)
                    nc.vector.tensor_add(x[:, m, :], ps[:, 0:NT], x[:, m, :])

            # ================= final LN + lm_head =================
            zf = ap_.tile([P, CCH, NT], f16, name="zf", tag="h")
            layernorm(x, zf)

            xf_outs = []
            for ha in range(2):
                xf_in = dp.tile([C, TKH], f16, name="xf_in",
                                tag=f"xf_in{ha}", bufs=1)
                xf_out = dp.tile([NCORES, C * TKH], f16, name="xf_out",
                                 tag=f"xf_out{ha}", bufs=1,
                                 addr_space="Shared")
                nc.sync.dma_start(
                    xf_in.rearrange("(c p) t -> p c t", p=P),
                    zf[:, :, ha * TKH:(ha + 1) * TKH])
                nc.gpsimd.collective_compute(
                    "AllGather", mybir.AluOpType.bypass,
                    replica_groups=groups8,
                    ins=[xf_in.opt()], outs=[xf_out.opt()])
                xf_outs.append(xf_out)

            # two passes: pass ha covers AG half ha = 1024 tokens (2 chunks
            # of 512); W_lm tiles are loaded once per pass.
            for ha in range(2):
                xf_n = []
                for rg in range(2):
                    xt = wp.tile([P, CCH, TKW], f16, name="xf_n",
                                 tag=f"xf{rg}", bufs=2)
                    for rl in range(GROUP):
                        nc.sync.dma_start(
                            xt[:, :, rl * TKH:(rl + 1) * TKH],
                            xf_outs[ha][rg * GROUP + rl]
                            .rearrange("(c p t) -> p c t", p=P, t=TKH))
                    xf_n.append(xt)
                for m in range(MLM):
                    wlm_t = wp.tile([P, CCH, MV], f16, name="wlm_t",
                                    tag="wA", bufs=8)
                    nc.sync.dma_start(wlm_t[:], d_wlm[m])
                    for rg in range(2):
                        n = 2 * ha + rg
                        ps = psp.tile([P, 2 * NT], f32, name="ps_lm",
                                      tag="mm", bufs=3)
                        for c in range(CCH):
                            nc.tensor.matmul(ps[0:MV, 0:TKW], wlm_t[:, c, :],
                                             xf_n[rg][:, c, :],
                                             start=(c == 0),
                                             stop=(c == CCH - 1))
                        lo = smp.tile([P, TKW], f32, name="lo", tag="lo",
                                      bufs=3)
                        nc.vector.tensor_scalar(lo[0:MV, :], ps[0:MV, 0:TKW],
                                                blm_sb[0:MV, m:m + 1], None,
                                                ALU.add)
                        nc.sync.dma_start(
                            d_out[m * MV:(m + 1) * MV,
                                  n * TKW:(n + 1) * TKW],
                            lo[0:MV, :])

    nc.compile()
    return nc


def _host_prep(inputs):
    """Fold LN scale/bias into weights, pre-tile lhsT weights, build masks."""
    f = np.float32
    h16 = np.float16
    g = {k: np.asarray(v) for k, v in inputs.items()}

    tok_emb = g["tok_emb"].astype(f)
    pos_emb = g["pos_emb"].astype(f)
    idx = np.asarray(g["idx"]).astype(np.int64)

    x0 = tok_emb[idx] + pos_emb[None, :T, :]          # [B, T, C]

    def cat_heads(w):                                  # [H, C, HS] -> [C, H*HS]
        return np.ascontiguousarray(w.transpose(1, 0, 2).reshape(C, H * HS))

    def tile_lhst(w, nm):
        # [Cin, Cout] -> [Cout/nm-chunks (m), P(p over Cin), Cin/P (c), f]
        cin = w.shape[0]
        r = w.reshape(cin // P, P, nm, w.shape[1] // nm)   # [c, p, m, f]
        return np.ascontiguousarray(
            r.transpose(2, 1, 0, 3).astype(h16))           # [m, p, c, f]

    wq = np.empty((L, CCH, P, CCH, P), h16)
    wk = np.empty((L, CCH, P, CCH, P), h16)
    wo = np.empty((L, CCH, P, CCH, P), h16)
    w1 = np.empty((L, FCH, P, CCH, P), h16)
    w2 = np.empty((L, CCH, P, FCH, P), h16)
    wv = np.empty((L, C, C), h16)
    qb = np.empty((L, P, CCH), f)
    kb = np.empty((L, P, CCH), f)
    b1t = np.empty((L, P, FCH), f)
    vb = np.empty((L, C), f)

    scale = 1.0 / np.sqrt(HS)
    for l in range(L):
        s1 = g["ln1_s"][l].astype(f)
        bn1 = g["ln1_b"][l].astype(f)
        s2 = g["ln2_s"][l].astype(f)
        bn2 = g["ln2_b"][l].astype(f)
        Wq = cat_heads(g["Wq"][l].astype(f))
        Wk = cat_heads(g["Wk"][l].astype(f))
        Wv = cat_heads(g["Wv"][l].astype(f))
        wq[l] = tile_lhst(s1[:, None] * Wq * scale, CCH)
        wk[l] = tile_lhst(s1[:, None] * Wk, CCH)
        wo[l] = tile_lhst(g["Wo"][l].astype(f), CCH)
        wv[l] = (s1[:, None] * Wv).astype(h16)
        qb[l] = ((bn1 @ Wq) * scale).reshape(CCH, P).T
        kb[l] = (bn1 @ Wk).reshape(CCH, P).T
        vb[l] = bn1 @ Wv
        W1 = g["W1"][l].astype(f)
        w1[l] = tile_lhst(s2[:, None] * W1, FCH)
        b1t[l] = (g["b1"][l].astype(f) + bn2 @ W1).reshape(FCH, P).T
        w2[l] = tile_lhst(g["W2"][l].astype(f), CCH)

    sf = g["lnf_s"].astype(f)
    bf = g["lnf_b"].astype(f)
    Wlm = g["W_lm"].astype(f)
    wlm_full = sf[:, None] * Wlm                      # [C, V]
    blm_full = (g["b_lm"].astype(f) + bf @ Wlm)       # [V]

    on16 = np.ones((P, NT), h16)
    id16 = np.eye(P, dtype=h16)

    shared = dict(
        wq=wq, wk=wk, wo=wo, w1=w1, w2=w2, wv=wv,
        qb=np.ascontiguousarray(qb), kb=np.ascontiguousarray(kb),
        b1=np.ascontiguousarray(b1t),
        bo=g["bo"].astype(h16), b2=g["b2"].astype(h16), vb=vb,
        on16=on16, id16=id16,
    )

    in_maps = []
    for core in range(NCORES):
        bb, cg = core // GROUP, core % GROUP
        x0t = np.ascontiguousarray(
            x0[bb, cg * NT:(cg + 1) * NT, :].T)       # [C, NT]
        qpos = cg * NT + np.arange(NT)
        kpos = np.arange(T)
        mask = np.where(kpos[:, None] <= qpos[None, :], 0.0, NEG).astype(h16)
        mask = mask.reshape(KB, P, NT)
        msk2 = np.ascontiguousarray(
            np.broadcast_to(mask[:, :, None, :], (KB, P, 2, NT)))
        wlm_s = wlm_full[:, core * VSH:(core + 1) * VSH]   # [C, 4000]
        blm_s = blm_full[core * VSH:(core + 1) * VSH]
        blm_t = np.zeros((P, MLM), f)
        blm_t[:MV, :] = blm_s.reshape(MLM, MV).T
        m = dict(shared)
        m["x0t"] = x0t
        m["msk"] = msk2
        m["wlm"] = tile_lhst(wlm_s, MLM)              # [32, 128, 8, 125]
        m["blm"] = blm_t
        in_maps.append(m)
    return in_maps


def _token_cols():
    """Global flat-token index for each column of the device output."""
    cols = np.empty(NCORES * NT, np.int64)
    i = 0
    for ha in range(2):
        for rg in range(2):
            for rl in range(GROUP):
                r = rg * GROUP + rl
                bb, cg = divmod(r, GROUP)
                base = bb * T + cg * NT + ha * TKH
                cols[i:i + TKH] = np.arange(base, base + TKH)
                i += TKH
    return cols


def kernel(**inputs):
    from concourse import bass_utils

    if "nc" not in _CACHE:
        _CACHE["nc"] = _build()
    nc = _CACHE["nc"]

    in_maps = _host_prep(inputs)
    trace = os.environ.get("BIGRAM_TRACE") == "1"
    res = bass_utils.run_bass_kernel_spmd(
        nc, in_maps, core_ids=list(range(NCORES)), trace=trace)
    _CACHE["last_res"] = res

    cols = _token_cols()
    out = np.empty((B * T, V), np.float32)
    for core in range(NCORES):
        out[cols, core * VSH:(core + 1) * VSH] = res.results[core]["logits"].T
    return out.reshape(B, T, V)
